# revision 1
# baseline (speedup 1.0000x reference)
"""Trainium2 Bass kernel for the logic-model log-likelihood.

Math (validated vs reference to ~1e-7 rel):
  raw = base + w*feat >= base > 0 always  =>  lam = raw (no branch).
  Integral term: per event closed-form geometric sum over the grid:
      C(t) = relu(exp(-D*(RES*(floor((t+TOL)/RES)+1) - t)) - exp(D*t - D*G*RES))
             / (1 - exp(-D*RES))
  log-sum term: exp(-D*(tq-te)) = exp(-D*tq)*exp(D*te), so
      k_q[s,h,eq,p] = exp(-D*tq) * sum_e (mask*exp(D*te)) * [tq - te > TOL]
  The compare matrix C[e,eq] (bf16 0/1) is built on DVE/GpSimd from a
  DMA-partition-broadcast of the query times; the masked sum is a PE matvec
  C^T @ A (bf16, FWL) accumulated over body pairs (h,p) in
  {(0,1),(0,2),(1,0),(2,0)}.

Sharding: data-parallel over samples S: 8 cores x 8 samples. Each core emits a
scalar partial; host sums the 8 partials (the gather/unshard step).
"""
import sys

import numpy as np

sys.path.insert(0, "/opt/trn_rl_repo")

import concourse.bacc as bacc
import concourse.mybir as mybir
from concourse import tile, library_config
from concourse.bass_utils import run_bass_kernel_spmd

F32 = mybir.dt.float32
I32 = mybir.dt.int32
BF16 = mybir.dt.bfloat16
AF = mybir.ActivationFunctionType
ALU = mybir.AluOpType

N_CORES = 8
S, P, E = 64, 3, 128
SC = S // N_CORES          # samples per core
ROWS = SC * P              # 24 (s,p) rows per core
DECAY, RES, TOL = 0.8, 0.03, 0.1
G = 1667                   # len(np.arange(0, 50, 0.03))
INV1MR = float(1.0 / (1.0 - np.exp(-DECAY * RES)))
BODY = np.array([[0, 1, 1], [1, 0, 0], [1, 0, 0]], dtype=np.float32)

# packed const block [128, 228]
_CB_ID, _CB_OC, _CB_OR, _CB_LM, _CB_SEL, _CB_CV = 0, 24, 25, 153, 177, 201
_CB_ID2 = 204
CVALS = [0.0, -DECAY * RES, -DECAY * G * RES]


def _const_block():
    cb = np.zeros((128, 228), np.float32)
    cb[0:ROWS, _CB_ID:_CB_ID + ROWS] = np.eye(ROWS)
    cb[:, _CB_OC] = 1.0
    cb[0, _CB_OR:_CB_OR + 128] = 1.0
    cb[0:P, _CB_LM:_CB_LM + ROWS] = np.tile(BODY, (1, SC))
    sel = np.zeros((P, ROWS), np.float32)
    for n in range(ROWS):
        sel[n % P, n] = 1.0
    cb[0:P, _CB_SEL:_CB_SEL + ROWS] = sel
    cb[:, _CB_CV:_CB_CV + len(CVALS)] = np.array(CVALS, np.float32)
    cb[32:32 + ROWS, _CB_ID2:_CB_ID2 + ROWS] = np.eye(ROWS)
    return cb


def _build_nc(reps=1):
    nc = bacc.Bacc(None, target_bir_lowering=False)
    em_d = nc.dram_tensor("em", [64, E], F32, kind="ExternalInput")
    wb_d = nc.dram_tensor("wb", [P, 2], F32, kind="ExternalInput")
    out_d = nc.dram_tensor("out", [1, 1], F32, kind="ExternalOutput")
    cb_d = nc.inline_tensor(_const_block(), "cblock")

    with tile.TileContext(nc) as tc:
        with (
            tc.tile_pool(name="const", bufs=1) as cpool,
            tc.tile_pool(name="inp", bufs=1) as ipool,
            tc.tile_pool(name="small", bufs=1) as spool,
            tc.tile_pool(name="cbuf", bufs=6) as cbpool,
            tc.tile_pool(name="psA", bufs=2, space="PSUM") as psA,
            tc.tile_pool(name="psFeat", bufs=1, space="PSUM") as psF,
            tc.tile_pool(name="psS", bufs=1, space="PSUM") as psS,
        ):
            nc.gpsimd.load_library(library_config.mlp)
            # ---- one DMA for all constants ----
            cblk = cpool.tile([128, 228], F32, tag="cblk")
            nc.sync.dma_start(cblk[:], cb_d[:])
            ident = cblk[0:ROWS, _CB_ID:_CB_ID + ROWS]
            ident2 = cblk[32:32 + ROWS, _CB_ID2:_CB_ID2 + ROWS]
            ones_col = cblk[:, _CB_OC:_CB_OC + 1]
            lmat = cblk[0:P, _CB_LM:_CB_LM + ROWS]
            for ci, cval in enumerate(CVALS):
                nc.const_aps.aps[(F32, cval)] = cblk[:, _CB_CV + ci:_CB_CV + ci + 1]

            def _body():
                # ---- inputs: one DMA for times+mask, one for w/b ----
                im = ipool.tile([64, E], F32, tag="im")
                nc.sync.dma_start(im[:], em_d[:])
                t_sb = im[0:ROWS, :]
                m_sb = im[32:32 + ROWS, :]
                wbt = ipool.tile([P, 2], F32, tag="wbt")
                nc.sync.dma_start(wbt[:], wb_d[:])
                w_col = wbt[:, 0:1]
                b_col = wbt[:, 1:2]

                # W24/B24 [128,2*24] via two matmuls: row pattern then bcast
                selm = cblk[0:P, _CB_SEL:_CB_SEL + ROWS]
                ones_row = cblk[0:1, _CB_OR:_CB_OR + 128]
                wbrow_ps = psS.tile([2, 2 * ROWS], F32, tag="wbrow")
                nc.tensor.matmul(wbrow_ps[0:1, 0:ROWS], wbt[:, 0:1], selm,
                                 start=True, stop=True)
                nc.tensor.matmul(wbrow_ps[0:1, ROWS:2 * ROWS], wbt[:, 1:2],
                                 selm, start=True, stop=True)
                wbrow = spool.tile([1, 2 * ROWS], F32, tag="wbrow_sb")
                nc.scalar.copy(wbrow[:], wbrow_ps[0:1, :])
                wb24_ps = psA.tile([128, 2 * ROWS], F32, tag="tr")
                nc.tensor.matmul(wb24_ps[:], ones_row, wbrow[:], start=True,
                                 stop=True)
                wb24 = spool.tile([128, 2 * ROWS], F32, tag="wb24")
                nc.scalar.copy(wb24[:], wb24_ps[:])
                w24 = wb24[:, 0:ROWS]
                b24 = wb24[:, ROWS:2 * ROWS]

                # ---- transposes: [24,128] -> [128,24] ----
                tT_ps = psA.tile([128, ROWS], F32, tag="tr")
                nc.tensor.transpose(tT_ps[:], t_sb, ident)
                t_T = spool.tile([128, ROWS], F32, tag="t_T")
                nc.scalar.copy(t_T[:], tT_ps[:])
                mT_ps = psA.tile([128, ROWS], F32, tag="tr")
                nc.tensor.transpose(mT_ps[:], m_sb, ident2)
                mask_T = spool.tile([128, ROWS], F32, tag="mask_T")
                nc.scalar.copy(mask_T[:], mT_ps[:])

                # ---- derived per-event tiles ----
                aexp = spool.tile([128, ROWS], F32, tag="aexp")
                nc.scalar.activation(aexp[:], t_T[:], AF.Exp, scale=DECAY)
                a_T = spool.tile([128, ROWS], BF16, tag="a_T")  # mask*exp(D*t)
                nc.vector.tensor_mul(a_T[:], aexp[:], mask_T[:])
                eq_T = spool.tile([128, ROWS], F32, tag="eq_T")  # exp(-D*t)
                nc.scalar.activation(eq_T[:], t_T[:], AF.Exp, scale=-DECAY)
                valid = spool.tile([128, ROWS], F32, tag="valid")
                nc.vector.tensor_copy(valid[:], mask_T[:])
                nc.vector.memset(valid[0:1, :], 0.0)

                # ---- log-sum term ----
                em_flat = em_d[:].rearrange("a b -> (a b)")
                feat_ps = psF.tile([128, ROWS], F32, tag="feat")
                for s in range(SC):
                    # broadcast this sample's 384 query times to all partitions
                    tq_s = cbpool.tile([128, P * E], F32, tag="tq_s")
                    nc.sync.dma_start(
                        tq_s[:],
                        em_flat[s * P * E:(s + 1) * P * E].partition_broadcast(128))
                    # C[e, eq] = (tq[eq] - te[e]) > TOL ; bf16 0/1 exact
                    # key p=0 serves heads 1,2 (cols E:3E); p=1,2 serve head 0
                    c0 = cbpool.tile([128, 2 * E], BF16, tag="c0")
                    nc.vector.tensor_scalar(
                        c0[:], tq_s[:, E:3 * E],
                        t_T[:, s * P:s * P + 1],
                        TOL, ALU.subtract, ALU.is_gt)
                    c1 = cbpool.tile([128, E], BF16, tag="c1")
                    nc.vector.tensor_scalar(
                        c1[:], tq_s[:, 0:E],
                        t_T[:, s * P + 1:s * P + 2],
                        TOL, ALU.subtract, ALU.is_gt)
                    c2 = cbpool.tile([128, E], BF16, tag="c2")
                    nc.vector.tensor_scalar(
                        c2[:], tq_s[:, 0:E],
                        t_T[:, s * P + 2:s * P + 3],
                        TOL, ALU.subtract, ALU.is_gt)
                    nc.tensor.matmul(
                        feat_ps[:, s * P:s * P + 1], c1[:],
                        a_T[:, s * P + 1:s * P + 2], start=True, stop=False)
                    nc.tensor.matmul(
                        feat_ps[:, s * P:s * P + 1], c2[:],
                        a_T[:, s * P + 2:s * P + 3], start=False, stop=True)
                    nc.tensor.matmul(
                        feat_ps[:, s * P + 1:s * P + 2], c0[:, 0:E],
                        a_T[:, s * P:s * P + 1], start=True, stop=True)
                    nc.tensor.matmul(
                        feat_ps[:, s * P + 2:s * P + 3], c0[:, E:2 * E],
                        a_T[:, s * P:s * P + 1], start=True, stop=True)

                # epilogue: lam = b + w*exp(-D*tq)*feat ; acc log(lam)*valid
                kq = spool.tile([128, ROWS], F32, tag="kq")
                nc.vector.tensor_mul(kq[:], feat_ps[:], eq_T[:])
                nc.vector.tensor_mul(kq[:], kq[:], w24)
                nc.vector.tensor_add(kq[:], kq[:], b24)
                nc.scalar.activation(kq[:], kq[:], AF.Ln)
                lnm = spool.tile([128, ROWS], F32, tag="lnm")
                nc.vector.tensor_mul(lnm[:], kq[:], valid[:])
                red_col = spool.tile([128, 1], F32, tag="red_col")
                nc.vector.reduce_sum(red_col[:], lnm[:], axis=mybir.AxisListType.X)
                ls_ps = psS.tile([1, 1], F32, tag="ls")
                nc.tensor.matmul(ls_ps[:], red_col[:], ones_col, start=True,
                                 stop=True)

                # ---- integral term: K_int[s*3+p] = sum_e mask*C(t) ----
                u_t = ipool.tile([ROWS, E], F32, tag="u_t")
                nc.vector.tensor_scalar(u_t[:], t_sb, TOL, 1.0 / RES,
                                        ALU.add, ALU.mult)
                ci_t = ipool.tile([ROWS, E], I32, tag="ci_t")
                nc.vector.tensor_copy(ci_t[:], u_t[:])
                cf_t = ipool.tile([ROWS, E], F32, tag="cf_t")
                nc.vector.tensor_copy(cf_t[:], ci_t[:])
                gt_t = ipool.tile([ROWS, E], F32, tag="gt_t")
                nc.vector.tensor_tensor(gt_t[:], cf_t[:], u_t[:], ALU.is_gt)
                nc.vector.tensor_sub(gt_t[:], cf_t[:], gt_t[:])
                nc.vector.tensor_scalar(gt_t[:], gt_t[:], RES, None, ALU.mult)
                x_t = ipool.tile([ROWS, E], F32, tag="x_t")
                nc.vector.tensor_sub(x_t[:], t_sb, gt_t[:])
                e1 = ipool.tile([ROWS, E], F32, tag="e1")
                nc.scalar.activation(e1[:], x_t[:], AF.Exp, scale=DECAY,
                                     bias=-DECAY * RES)
                e2 = ipool.tile([ROWS, E], F32, tag="e2")
                nc.scalar.activation(e2[:], t_sb, AF.Exp, scale=DECAY,
                                     bias=-DECAY * G * RES)
                dmr = ipool.tile([ROWS, E], F32, tag="dmr")
                nc.vector.tensor_sub(dmr[:], e1[:], e2[:])
                nc.scalar.activation(dmr[:], dmr[:], AF.Relu)
                msk0 = ipool.tile([ROWS, E], F32, tag="msk0")
                nc.scalar.copy(msk0[:], m_sb)
                cm = ipool.tile([ROWS, E], F32, tag="cm")
                nc.vector.tensor_mul(cm[:], dmr[:], msk0[:])
                k_int = spool.tile([ROWS, 1], F32, tag="k_int")
                nc.vector.reduce_sum(k_int[:], cm[:], axis=mybir.AxisListType.X)

                # v[s*3+p] = sum_h body[h,p]*w[h]; kdot = v . K_int
                v_ps = psS.tile([ROWS, 1], F32, tag="v_ps")
                nc.tensor.matmul(v_ps[:], lmat, w_col, start=True, stop=True)
                v_sb = spool.tile([ROWS, 1], F32, tag="v_sb")
                nc.scalar.copy(v_sb[:], v_ps[:])
                kdot_ps = psS.tile([1, 1], F32, tag="kdot")
                nc.tensor.matmul(kdot_ps[:], v_sb[:], k_int[:], start=True,
                                 stop=True)
                bsum_ps = psS.tile([1, 1], F32, tag="bsum")
                nc.tensor.matmul(bsum_ps[:], b_col, ones_col[0:P, :],
                                 start=True, stop=True)

                # ---- final: out = logsum - RES*INV1MR*kdot - RES*SC*G*bsum
                fin = spool.tile([1, 1], F32, tag="fin")
                nc.scalar.mul(fin[:], kdot_ps[:], -RES * INV1MR)
                fin2 = spool.tile([1, 1], F32, tag="fin2")
                nc.scalar.mul(fin2[:], bsum_ps[:], -RES * SC * G)
                nc.vector.tensor_add(fin[:], fin[:], fin2[:])
                nc.vector.tensor_add(fin[:], fin[:], ls_ps[:])
                nc.sync.dma_start(out_d[:], fin[:])

            if reps == 1:
                _body()
            else:
                with tc.For_i(0, reps, 1):
                    _body()

    nc.compile()
    return nc


_NC = None


def _get_nc():
    global _NC
    if _NC is None:
        _NC = _build_nc()
    return _NC


def make_in_maps(event_times, event_mask, base, weight):
    et = np.ascontiguousarray(np.asarray(event_times, np.float32))
    mk = np.ascontiguousarray(np.asarray(event_mask, np.float32))
    wb = np.stack([np.asarray(weight, np.float32).reshape(P),
                   np.asarray(base, np.float32).reshape(P)], axis=1).copy()
    in_maps = []
    for c in range(N_CORES):
        em = np.zeros((64, E), np.float32)
        em[0:ROWS] = et[c * SC:(c + 1) * SC].reshape(ROWS, E)
        em[32:32 + ROWS] = mk[c * SC:(c + 1) * SC].reshape(ROWS, E)
        in_maps.append({"em": em, "wb": wb})
    return in_maps


LAST_RESULT = None


def kernel(event_times, event_mask, base, weight, T_max=50, _trace=False, **_):
    global LAST_RESULT
    nc = _get_nc()
    in_maps = make_in_maps(event_times, event_mask, base, weight)
    kwargs = {}
    if _trace:
        kwargs = dict(trace=True, trace_cores=list(range(N_CORES)))
    res = run_bass_kernel_spmd(nc, in_maps, core_ids=list(range(N_CORES)),
                               **kwargs)
    LAST_RESULT = res
    total = np.float32(0.0)
    for r in res.results:
        total += np.float32(r["out"][0, 0])
    return np.asarray(total, dtype=np.float32)


def run_timing(in_maps, reps, core_ids=None):
    """Module with the body repeated `reps` times on-device (For_i loop);
    used for loop-amortized wall-clock HW timing."""
    nc = _build_nc(reps=reps)
    if core_ids is None:
        core_ids = list(range(N_CORES))
    return run_bass_kernel_spmd(nc, in_maps, core_ids=core_ids)



# revision 6
# speedup vs baseline: 1.0486x; 1.0486x over previous
"""Trainium2 Bass kernel for the logic-model log-likelihood (v3).

Changes vs v2 (driven by the v2 NTFF trace):
  - No dummy activation and no const-block bias columns: the framework's
    own memset consts serve activation biases, so nothing on the Act
    engine waits for the const DMA. The single (rewritten) act-table
    load sits first in the Act program and runs at t~0.
  - Query times are broadcast as fp16 (half the DMA bytes; host supplies
    the fp16 copy), and compares run as 24 tensor_scalar ops in DVE 2x
    mode, interleaved per-sample with the matvec matmuls.
  - DMAs: small inputs first, split across SP/Pool/DVE issue queues.
  - Integral ALU chain runs on the otherwise-idle GpSimd engine.
  - Epilogue: eqd is pre-masked with the valid mask, so dead cells give
    ln(b); the host subtracts sum((384-V)*ln(b)) exactly. Path after the
    last matmul is just mult -> Ln(accum) -> 2 tiny matmuls -> DMA (from
    PSUM directly).
"""
import sys

import numpy as np

sys.path.insert(0, "/opt/trn_rl_repo")

import concourse.bacc as bacc
import concourse.mybir as mybir
from concourse import tile
from concourse.bass_utils import run_bass_kernel_spmd

F32 = mybir.dt.float32
F16 = mybir.dt.float16
I32 = mybir.dt.int32
BF16 = mybir.dt.bfloat16
AF = mybir.ActivationFunctionType
ALU = mybir.AluOpType

N_CORES = 8
S, P, E = 64, 3, 128
SC = S // N_CORES          # samples per core
ROWS = SC * P              # 24 (s,p) rows per core
DECAY, RES, TOL = 0.8, 0.03, 0.1
G = 1667                   # len(np.arange(0, 50, 0.03))
INV1MR = float(1.0 / (1.0 - np.exp(-DECAY * RES)))
E2C = float(np.exp(-DECAY * G * RES))
BODY = np.array([[0, 1, 1], [1, 0, 0], [1, 0, 0]], dtype=np.float32)

# natural_log_exp_and_others: exp, ln, copy, relu in one table
_ACT_SET_ALL = 6


def _cmini():
    cb = np.zeros((128, ROWS + 2), np.float32)
    cb[0:ROWS, 0:ROWS] = np.eye(ROWS)
    cb[:, ROWS] = 1.0
    cb[:, ROWS + 1] = -DECAY * RES
    return cb


def _bdiag():
    bd = np.zeros((SC, ROWS), np.float32)
    for s in range(SC):
        bd[s, 3 * s:3 * s + 3] = 1.0
    return np.broadcast_to(bd.reshape(1, SC * ROWS), (128, SC * ROWS)).copy()


def _rdiag():
    rd = np.zeros((ROWS, P, E), np.float32)
    for r in range(ROWS):
        rd[r, r % 3, :] = 1.0
    return rd.reshape(ROWS, P * E)


def _build_nc():
    nc = bacc.Bacc(None, target_bir_lowering=False)
    em_d = nc.dram_tensor("em", [ROWS, E], F32, kind="ExternalInput")
    t16_d = nc.dram_tensor("t16", [ROWS, E], F16, kind="ExternalInput")
    mk_d = nc.dram_tensor("mk", [ROWS, E], F32, kind="ExternalInput")
    wbx_d = nc.dram_tensor("wbx", [ROWS, 4], F32, kind="ExternalInput")
    out_d = nc.dram_tensor("out", [1, 1], F32, kind="ExternalOutput")
    cm_d = nc.inline_tensor(_cmini(), "cmini")
    bd_d = nc.inline_tensor(_bdiag(), "bdiag")
    rd_d = nc.inline_tensor(_rdiag(), "rdiag")
    t16_flat = t16_d[:].rearrange("a b -> (a b)")

    with tile.TileContext(nc) as tc:
        with (
            tc.tile_pool(name="const", bufs=1) as cpool,
            tc.tile_pool(name="inp", bufs=1) as ipool,
            tc.tile_pool(name="tq", bufs=1) as qpool,
            tc.tile_pool(name="cmp", bufs=1) as cmpool,
            tc.tile_pool(name="work", bufs=1) as wpool,
            tc.tile_pool(name="psT", bufs=1, space="PSUM") as psT,
            tc.tile_pool(name="psA", bufs=1, space="PSUM") as psA,
            tc.tile_pool(name="psK", bufs=1, space="PSUM") as psK,
            tc.tile_pool(name="psI", bufs=1, space="PSUM") as psI,
            tc.tile_pool(name="psD", bufs=1, space="PSUM") as psD,
        ):
            # ---- DMAs: small inputs first, spread across issue queues ----
            im = ipool.tile([ROWS, E], F32, tag="im")
            nc.sync.dma_start(im[:], em_d[:])
            tq16 = qpool.tile([128, SC * P * E], F16, tag="tq16")
            nc.sync.dma_start(
                tq16[:, 0:4 * P * E],
                t16_flat[0:4 * P * E].partition_broadcast(128))
            nc.sync.dma_start(
                tq16[:, 4 * P * E:8 * P * E],
                t16_flat[4 * P * E:8 * P * E].partition_broadcast(128))
            mm_t = ipool.tile([ROWS, E], F32, tag="mm_t")
            nc.gpsimd.dma_start(mm_t[:], mk_d[:])
            cmini = cpool.tile([128, ROWS + 2], F32, tag="cmini")
            nc.gpsimd.dma_start(cmini[:], cm_d[:])
            bdiag = cpool.tile([128, SC * ROWS], F32, tag="bdiag")
            nc.gpsimd.dma_start(bdiag[:], bd_d[:])
            rdiag = cpool.tile([ROWS, P * E], F32, tag="rdiag")
            nc.scalar.dma_start(rdiag[:], rd_d[:])
            wbx = wpool.tile([ROWS, 4], F32, tag="wbx")
            nc.scalar.dma_start(wbx[:], wbx_d[:])

            t_sb = im[:]
            m_sb = mm_t[:]
            eye24 = cmini[0:ROWS, 0:ROWS]
            ones_col = cmini[:, ROWS:ROWS + 1]
            nc.const_aps.aps[(F32, -DECAY * RES)] = \
                cmini[:, ROWS + 1:ROWS + 2]
            w_col = wbx[:, 0:1]
            b_col = wbx[:, 1:2]
            v_col = wbx[:, 3:4]

            # ---- prep ----
            aexp = wpool.tile([ROWS, E], F32, tag="aexp")
            nc.scalar.activation(aexp[:], t_sb, AF.Exp, scale=DECAY)
            a_sb = wpool.tile([ROWS, E], F32, tag="a_sb")
            nc.vector.tensor_mul(a_sb[:], aexp[:], m_sb)

            t_ps = psT.tile([128, ROWS], F32, tag="t_ps")
            nc.tensor.transpose(t_ps[:], t_sb, eye24)
            tp_T = wpool.tile([128, ROWS], F32, tag="tp_T")  # t^T + TOL
            nc.vector.tensor_scalar(tp_T[:], t_ps[:], TOL, None, ALU.add)

            a_ps = psA.tile([128, ROWS], F32, tag="a_ps")
            nc.tensor.transpose(a_ps[:], a_sb[:], eye24)
            a_T = wpool.tile([128, ROWS + 2], BF16, tag="a_T")
            nc.vector.tensor_copy(a_T[:, 0:ROWS], a_ps[:])
            nc.vector.memset(a_T[:, ROWS:ROWS + 2], 0.0)

            # S1 gather: col 3s <- a[s,1]; cols 3s+1,3s+2 <- a[s,0]
            s1 = wpool.tile([128, ROWS + 1], BF16, tag="s1")
            nc.vector.tensor_copy(s1[:, 0:ROWS:3], a_T[:, 1:ROWS:3])
            nc.vector.tensor_copy(
                s1[:, 1:ROWS + 1].rearrange("p (a b) -> p a b", b=3)[:, :, 0:2],
                a_T[:, 0:ROWS:3][:, :, None].broadcast_to([128, SC, 2]))
            # block-diag masked stationaries [128, 192]
            s1_big = wpool.tile([128, SC * ROWS], BF16, tag="s1_big")
            nc.vector.tensor_mul(
                s1_big[:].rearrange("p (a b) -> p a b", b=ROWS),
                s1[:, 0:ROWS][:, None, :].broadcast_to([128, SC, ROWS]),
                bdiag[:].rearrange("p (a b) -> p a b", b=ROWS))
            s2_big = wpool.tile([128, SC * ROWS], BF16, tag="s2_big")
            nc.vector.tensor_mul(
                s2_big[:].rearrange("p (a b) -> p a b", b=ROWS),
                a_T[:, 2:ROWS + 2][:, None, :].broadcast_to([128, SC, ROWS]),
                bdiag[:].rearrange("p (a b) -> p a b", b=ROWS))

            # valid mask: zero the event-0 column (after a_sb consumed m_sb)
            nc.vector.memset(mm_t[:, 0:1], 0.0)

            # eqd_m[(s,h), 128h+e] = exp(-D*t[s,h,e]) * valid, 0 off-diagonal
            eqd = wpool.tile([ROWS, P * E], F32, tag="eqd")
            nc.scalar.activation(
                eqd[:].rearrange("p (a b) -> p a b", b=E),
                t_sb[:, None, :].broadcast_to([ROWS, P, E]),
                AF.Exp, scale=-DECAY)
            vd = wpool.tile([ROWS, P * E], F32, tag="vd")
            nc.vector.tensor_mul(
                vd[:].rearrange("p (a b) -> p a b", b=E),
                m_sb[:, None, :].broadcast_to([ROWS, P, E]),
                rdiag[:].rearrange("p (a b) -> p a b", b=E))
            nc.vector.tensor_mul(eqd[:], eqd[:], vd[:])

            # ---- integral ALU chain on GpSimd (idle engine) ----
            u = wpool.tile([128, ROWS], F32, tag="u")
            nc.vector.tensor_scalar(u[:], tp_T[:], 1.0 / RES, None, ALU.mult)
            ci_t = wpool.tile([128, ROWS], I32, tag="ci_t")
            nc.vector.tensor_copy(ci_t[:], u[:])
            cf_t = wpool.tile([128, ROWS], F32, tag="cf_t")
            nc.vector.tensor_copy(cf_t[:], ci_t[:])
            gt_t = wpool.tile([128, ROWS], F32, tag="gt_t")
            nc.vector.tensor_tensor(gt_t[:], cf_t[:], u[:], ALU.is_gt)
            nc.vector.tensor_sub(gt_t[:], cf_t[:], gt_t[:])  # floor(u)
            ie = wpool.tile([128, ROWS], F32, tag="ie")
            nc.scalar.activation(ie[:], gt_t[:], AF.Exp, scale=-DECAY * RES,
                                 bias=-DECAY * RES)
            nc.vector.tensor_scalar(ie[:], ie[:], E2C, 0.0, ALU.subtract,
                                    ALU.max)
            cm = wpool.tile([128, ROWS], F32, tag="cm")
            nc.vector.tensor_mul(cm[:], ie[:], a_ps[:])
            kint_ps = psI.tile([ROWS, 1], F32, tag="kint")
            nc.tensor.matmul(kint_ps[:], cm[:], ones_col, start=True, stop=True)
            kint_sb = wpool.tile([ROWS, 1], F32, tag="kint_sb")
            nc.vector.tensor_copy(kint_sb[:], kint_ps[:])
            kdot_ps = psD.tile([1, 1], F32, tag="kdot")
            nc.tensor.matmul(kdot_ps[:], kint_sb[:], v_col, start=True,
                             stop=False, skip_group_check=True)

            # ---- compares + matvecs, interleaved per sample ----
            # per-sample 512 col block of C: [c1 | c0a c0b | c2]
            call = cmpool.tile([128, SC * 4 * E], BF16, tag="call")
            kk_ps = psK.tile([ROWS, P * E], F32, tag="kk")

            for s in range(SC):
                base = 4 * E * s
                q0 = tq16[:, P * E * s:P * E * s + E]        # pred-0 queries
                q12 = tq16[:, P * E * s + E:P * E * (s + 1)]  # pred-1,2 queries
                nc.vector.tensor_scalar(
                    call[:, base:base + E], q0,
                    tp_T[:, 3 * s + 1:3 * s + 2], 0.0, ALU.subtract, ALU.is_gt)
                nc.vector.tensor_scalar(
                    call[:, base + E:base + 3 * E], q12,
                    tp_T[:, 3 * s:3 * s + 1], 0.0, ALU.subtract, ALU.is_gt)
                nc.vector.tensor_scalar(
                    call[:, base + 3 * E:base + 4 * E], q0,
                    tp_T[:, 3 * s + 2:3 * s + 3], 0.0, ALU.subtract, ALU.is_gt)
                nc.tensor.matmul(
                    kk_ps[:], s1_big[:, ROWS * s:ROWS * (s + 1)],
                    call[:, base:base + 3 * E],
                    start=(s == 0), stop=False, skip_group_check=True)
                nc.tensor.matmul(
                    kk_ps[:, 0:E], s2_big[:, ROWS * s:ROWS * (s + 1)],
                    call[:, base + 3 * E:base + 4 * E],
                    start=False, stop=(s == SC - 1), skip_group_check=True)

            # ---- epilogue: keq -> ln(w*keq+b) with free accum ----
            keq = wpool.tile([ROWS, P * E], F32, tag="keq")
            nc.vector.tensor_mul(keq[:], kk_ps[:], eqd[:])
            lnr = wpool.tile([ROWS, P * E], F32, tag="lnr")
            acc = wpool.tile([ROWS, 1], F32, tag="acc")
            nc.scalar.activation(lnr[:], keq[:], AF.Ln, bias=b_col,
                                 scale=w_col, accum_out=acc[:])
            # ls + kdot accumulate in one PSUM cell
            nc.tensor.matmul(kdot_ps[:], acc[:], ones_col[0:ROWS, :],
                             start=False, stop=True, skip_group_check=True)
            fin = wpool.tile([1, 1], F32, tag="fin")
            nc.vector.tensor_copy(fin[:], kdot_ps[:])
            nc.sync.dma_start(out_d[:], fin[:])

    nc.compile()
    _unify_act_tables(nc)
    return nc


def _unify_act_tables(nc):
    for blk in nc.m.functions[0].blocks:
        loads = [i for i in blk.instructions
                 if isinstance(i, mybir.InstLoadActFuncSet)]
        if not loads:
            continue
        loads[0].act_func_set_id = _ACT_SET_ALL
        for ins in loads[1:]:
            blk.instructions.remove(ins)


_NC = None


def _get_nc():
    global _NC
    if _NC is None:
        _NC = _build_nc()
    return _NC


def make_in_maps(event_times, event_mask, base, weight):
    et = np.ascontiguousarray(np.asarray(event_times, np.float32))
    mk = np.ascontiguousarray(np.asarray(event_mask, np.float32))
    w = np.asarray(weight, np.float32).reshape(P)
    b = np.asarray(base, np.float32).reshape(P)
    wbx = np.zeros((ROWS, 4), np.float32)
    wbx[:, 0] = np.tile(w, SC)
    wbx[:, 1] = np.tile(b, SC)
    v = -RES * INV1MR * (BODY.T @ w)          # [P]
    wbx[:, 3] = np.tile(v, SC)
    in_maps = []
    for c in range(N_CORES):
        em = et[c * SC:(c + 1) * SC].reshape(ROWS, E).copy()
        mm = mk[c * SC:(c + 1) * SC].reshape(ROWS, E).copy()
        in_maps.append({"em": em, "t16": em.astype(np.float16),
                        "mk": mm, "wbx": wbx})
    return in_maps


def host_const(event_mask, base):
    """-RES*G*S*sum(b)  minus the ln(b) contributions of dead cells."""
    b = np.asarray(base, np.float64).reshape(P)
    mk = np.asarray(event_mask, np.float64)
    v_cnt = mk[:, :, 1:].sum(axis=2)              # [S, P] valid counts
    junk = ((P * E - v_cnt) * np.log(b)[None, :]).sum()
    return float(-RES * G * S * b.sum() - junk)


LAST_RESULT = None


def kernel(event_times, event_mask, base, weight, T_max=50, _trace=False, **_):
    global LAST_RESULT
    nc = _get_nc()
    in_maps = make_in_maps(event_times, event_mask, base, weight)
    kwargs = {}
    if _trace:
        kwargs = dict(trace=True, trace_cores=list(range(N_CORES)))
    res = run_bass_kernel_spmd(nc, in_maps, core_ids=list(range(N_CORES)),
                               **kwargs)
    LAST_RESULT = res
    total = np.float64(0.0)
    for r in res.results:
        total += float(r["out"][0, 0])
    total += host_const(event_mask, base)
    return np.asarray(total, dtype=np.float32)
